# revision 11
# baseline (speedup 1.0000x reference)
"""ChebConv (K=4) Trainium2 kernel: 8-core row-sharded SpMM + dense contraction.

Dataflow per core (rows [c*6250, (c+1)*6250) padded to 6272):
  x0 table (50176, 256) f32 in DRAM (host-built, padded global ids).
  3 Chebyshev SpMM steps: per 128-row tile, dma_gather the edge columns'
  z-rows from the table (int16 idx, lo/hi split at 32768), then reduce into
  the tile's 128 rows with per-128-slot indicator matmuls on the PE
  (lhsT M[slot, row] = val*(row_local[slot]==row), fp32r), accumulating in
  PSUM. Recurrence x_k = (2L)x_{k-1} - x_{k-2} via DVE subtract against the
  SBUF-resident ping/pong slice. Slices are AllGather'd into the next step's
  full table. Contraction phase: PE-transpose x_k tiles, then
  out[b].T (Cout, v) = sum_k W_k.T @ x_k[b].T with stationary fp32r weights.
"""

import sys

sys.path.insert(0, "/opt/trn_rl_repo")

import numpy as np

V = 50000
E = 800000
B, CIN, COUT, K = 4, 64, 128, 4
NC = 8
VC = V // NC              # 6250
VCP = 6272                # 49*128 padded rows per core
TILES = VCP // 128        # 49
VPT = NC * VCP            # 50176 table rows
F = B * CIN               # 256
HI_BASE = 32768           # int16 index split


# ---------------- host-side preprocessing ----------------

def _remap_col(g):
    return (g // VC) * VCP + (g % VC)


def preprocess(rows, cols, vals):
    """Split edges per core/tile into lo/hi index halves, pad to uniform
    counts, emit gather indices + per-chunk indicator metadata."""
    rows = np.asarray(rows)
    cols_r = _remap_col(np.asarray(cols).astype(np.int64))
    vals = np.asarray(vals, dtype=np.float32)

    per_core = []
    max_lo = max_hi = 0
    for c in range(NC):
        lo_r, hi_r = c * VC, (c + 1) * VC
        m = (rows >= lo_r) & (rows < hi_r)
        r = rows[m] - lo_r
        cg = cols_r[m]
        vv = vals[m]
        t_of = r // 128
        tiles = []
        for t in range(TILES):
            sel = t_of == t
            rt, ct, vt = r[sel], cg[sel], vv[sel]
            isl = ct < HI_BASE
            tl = (rt[isl] - t * 128, ct[isl], vt[isl])
            th = (rt[~isl] - t * 128, ct[~isl] - HI_BASE, vt[~isl])
            max_lo = max(max_lo, len(tl[0]))
            max_hi = max(max_hi, len(th[0]))
            tiles.append((tl, th))
        per_core.append(tiles)

    NLO = -(-max_lo // 128) * 128
    NHI = -(-max_hi // 128) * 128
    CPT = (NLO + NHI) // 128          # chunks per tile
    NCH = TILES * CPT
    SPT16 = (NLO + NHI) // 16         # idx columns per tile

    cores = []
    for c in range(NC):
        idx = np.zeros((128, TILES * SPT16), dtype=np.int16)
        rl = np.zeros((128, NCH), dtype=np.float32)
        v1 = np.zeros((128, NCH), dtype=np.float32)
        for t in range(TILES):
            (rlo, clo, vlo), (rhi, chi, vhi) = per_core[c][t]
            # slot order within tile: lo edges, lo pads, hi edges, hi pads
            rr = np.zeros(NLO + NHI, np.float32)
            cc = np.zeros(NLO + NHI, np.int32)
            vv = np.zeros(NLO + NHI, np.float32)
            n = len(rlo)
            rr[:n], cc[:n], vv[:n] = rlo, clo, vlo
            nh = len(rhi)
            rr[NLO : NLO + nh], cc[NLO : NLO + nh], vv[NLO : NLO + nh] = rhi, chi, vhi
            # gather idx, 16-wrapped, replicated across the 8 Q7 groups
            w = cc.reshape(SPT16, 16).T.astype(np.int16)       # (16, SPT16)
            idx[:, t * SPT16 : (t + 1) * SPT16] = np.tile(w, (8, 1))
            # chunk metadata: slot i -> chunk i//128, partition i%128
            ch0 = t * CPT
            rl[:, ch0 : ch0 + CPT] = rr.reshape(CPT, 128).T
            v1[:, ch0 : ch0 + CPT] = vv.reshape(CPT, 128).T
        cores.append(dict(idx=idx, rl=rl, v1=v1, v2=2.0 * v1))
    return cores, NLO, NHI


def host_inputs(x, lap_rows, lap_cols, lap_vals, weight, bias):
    x0 = np.ascontiguousarray(np.transpose(x, (2, 0, 1)).reshape(V, F)).astype(np.float32)
    table0 = np.zeros((VPT, F), dtype=np.float32)
    for c in range(NC):
        table0[c * VCP : c * VCP + VC] = x0[c * VC : (c + 1) * VC]
    cores, NLO, NHI = preprocess(lap_rows, lap_cols, lap_vals)
    iota = np.broadcast_to(np.arange(128, dtype=np.float32)[None, :], (128, 128)).copy()
    wlo = np.zeros((128, K * COUT), np.float32)
    whi = np.zeros((128, K * COUT), np.float32)
    for k in range(K):
        wlo[0:64, k * COUT : (k + 1) * COUT] = weight[k]
        whi[64:128, k * COUT : (k + 1) * COUT] = weight[k]
    bias_t = np.asarray(bias, np.float32).reshape(128, 1)
    in_maps = []
    for c in range(NC):
        in_maps.append(
            dict(
                table0=table0,
                x0slice=np.ascontiguousarray(table0[c * VCP : (c + 1) * VCP]),
                idx=cores[c]["idx"],
                rl=cores[c]["rl"],
                v1=cores[c]["v1"],
                v2=cores[c]["v2"],
                iota=iota,
                wlo=wlo,
                whi=whi,
                bias=bias_t,
            )
        )
    return in_maps, NLO, NHI


# ---------------- device module ----------------

_CACHE = {}


def build_module(NLO, NHI):
    key = (NLO, NHI)
    if key in _CACHE:
        return _CACHE[key]
    from concourse import bass, mybir, bacc
    import concourse.tile as tile
    from concourse.masks import make_identity

    CPT = (NLO + NHI) // 128
    NCH = TILES * CPT
    SPT16 = (NLO + NHI) // 16
    BLO = NLO // 128
    f32, f32r, i16 = mybir.dt.float32, mybir.dt.float32r, mybir.dt.int16

    nc = bacc.Bacc("TRN2", target_bir_lowering=False, debug=False, num_devices=NC)

    table0 = nc.dram_tensor("table0", [VPT, F], f32, kind="ExternalInput")
    x0slice = nc.dram_tensor("x0slice", [VCP, F], f32, kind="ExternalInput")
    idx_in = nc.dram_tensor("idx", [128, TILES * SPT16], i16, kind="ExternalInput")
    rl_in = nc.dram_tensor("rl", [128, NCH], f32, kind="ExternalInput")
    v1_in = nc.dram_tensor("v1", [128, NCH], f32, kind="ExternalInput")
    v2_in = nc.dram_tensor("v2", [128, NCH], f32, kind="ExternalInput")
    iota_in = nc.dram_tensor("iota", [128, 128], f32, kind="ExternalInput")
    wlo_in = nc.dram_tensor("wlo", [128, K * COUT], f32, kind="ExternalInput")
    whi_in = nc.dram_tensor("whi", [128, K * COUT], f32, kind="ExternalInput")
    bias_in = nc.dram_tensor("bias", [128, 1], f32, kind="ExternalInput")
    out_t = nc.dram_tensor("out", [B, COUT, VCP], f32, kind="ExternalOutput")

    with tile.TileContext(nc) as tc:
        with (
            tc.tile_pool(name="pers", bufs=1) as pers,
            tc.tile_pool(name="gpool", bufs=2) as gpool,
            tc.tile_pool(name="mval", bufs=4) as mvpool,
            tc.tile_pool(name="spmm_ps", bufs=2, space="PSUM") as pspool,
            tc.tile_pool(name="tp_ps", bufs=2, space="PSUM") as tppool,
            tc.tile_pool(name="out_ps", bufs=2, space="PSUM") as popool,
            tc.tile_pool(name="stage", bufs=3) as spool,
            tc.tile_pool(name="xt", bufs=5) as xtpool,
            tc.tile_pool(name="obuf", bufs=3) as obpool,
            tc.tile_pool(name="dram", bufs=1, space="DRAM") as dram,
        ):
            # persistent loads
            idx_t = pers.tile([128, TILES * SPT16], i16)
            nc.sync.dma_start(idx_t[:], idx_in[:])
            rl_t = pers.tile([128, NCH], f32)
            nc.sync.dma_start(rl_t[:], rl_in[:])
            v1_t = pers.tile([128, NCH], f32)
            nc.sync.dma_start(v1_t[:], v1_in[:])
            v2_t = pers.tile([128, NCH], f32)
            nc.sync.dma_start(v2_t[:], v2_in[:])
            iota_t = pers.tile([128, 128], f32)
            nc.sync.dma_start(iota_t[:], iota_in[:])
            wlo_t = pers.tile([128, K * COUT], f32r)
            nc.gpsimd.dma_start(wlo_t[:], wlo_in[:])
            whi_t = pers.tile([128, K * COUT], f32r)
            nc.gpsimd.dma_start(whi_t[:], whi_in[:])
            bias_t = pers.tile([128, 1], f32)
            nc.sync.dma_start(bias_t[:], bias_in[:])
            ident = pers.tile([128, 128], f32)
            make_identity(nc, ident[:])

            bounce = [dram.tile([VCP, F], f32, name=f"bounce{i}", tag=f"bounce{i}") for i in range(3)]
            tables = [dram.tile([VPT, F], f32, name=f"table{i+1}", tag=f"table{i+1}") for i in range(2)]

            # ---------- SpMM steps ----------
            for k in (1, 2, 3):
                src = table0 if k == 1 else tables[k - 2]
                vmeta = v1_t if k == 1 else v2_t
                prev_src = None if k == 1 else (x0slice if k == 2 else bounce[0])
                for t in range(TILES):
                    gt = gpool.tile([128, CPT * F], f32r, tag="G")
                    c0 = t * SPT16
                    # Q7 scratch caps one gather at ~1024 idxs; split into <=896 segs
                    segs = []
                    for base, n, hi in ((0, NLO, False), (BLO * 128, NHI, True)):
                        done = 0
                        while done < n:
                            m = min(896, n - done)
                            segs.append((base + done, m, hi))
                            done += m
                    for off, n, hi in segs:
                        sap = src[HI_BASE:, :] if hi else src[:]
                        nc.gpsimd.dma_gather(
                            out_ap=gt[:, off * 2 : (off + n) * 2].rearrange(
                                "p (j f) -> p j f", f=F),
                            in_ap=sap.bitcast(f32r),
                            idxs_ap=idx_t[:, c0 + off // 16 : c0 + (off + n) // 16],
                            num_idxs=n, num_idxs_reg=n, elem_size=F,
                            single_packet=False,
                        )
                    ps = pspool.tile([128, F], mybir.dt.float32, space="PSUM")
                    for j in range(CPT):
                        ch = t * CPT + j
                        mv = mvpool.tile([128, 128], f32r)
                        nc.any.tensor_scalar(
                            out=mv[:], in0=iota_t[:],
                            scalar1=rl_t[:, ch : ch + 1], scalar2=vmeta[:, ch : ch + 1],
                            op0=mybir.AluOpType.is_equal, op1=mybir.AluOpType.mult,
                        )
                        nc.tensor.matmul(
                            out=ps[:], lhsT=mv[:], rhs=gt[:, j * F : (j + 1) * F],
                            start=(j == 0), stop=(j == CPT - 1),
                        )
                    xo = spool.tile([128, F], f32, tag="xout")
                    if k == 1:
                        nc.any.tensor_copy(out=xo[:], in_=ps[:])
                    else:
                        xp = spool.tile([128, F], f32, tag="xprev")
                        nc.sync.dma_start(xp[:], prev_src[t * 128 : (t + 1) * 128, :])
                        nc.vector.tensor_tensor(out=xo[:], in0=ps[:], in1=xp[:], op=mybir.AluOpType.subtract)
                    nc.sync.dma_start(bounce[k - 1][t * 128 : (t + 1) * 128, :], xo[:])
                if k < 3:
                    nc.gpsimd.collective_compute(
                        "AllGather", mybir.AluOpType.bypass,
                        replica_groups=[list(range(NC))],
                        ins=[bounce[k - 1].opt()], outs=[tables[k - 1].opt()],
                    )

            # ---------- contraction ----------
            srcs = [x0slice, bounce[0], bounce[1], bounce[2]]
            vblocks = [(i * 512, 512) for i in range(VCP // 512)]
            if VCP % 512:
                vblocks.append((VCP // 512 * 512, VCP % 512))
            for v0, nv in vblocks:
                nq = nv // 128
                xts = []
                for k in range(K):
                    stage = spool.tile([128, 4 * F], f32, tag="stage")
                    nc.sync.dma_start(
                        stage[:, : nq * F].rearrange("p (q f) -> p q f", f=F),
                        srcs[k][v0 : v0 + nv, :].rearrange("(q p) f -> p q f", p=128),
                    )
                    xt_lo = xtpool.tile([128, 512], f32r, tag="xtlo")
                    xt_hi = xtpool.tile([128, 512], f32r, tag="xthi")
                    for q in range(nq):
                        for h in range(2):
                            tp = tppool.tile([128, 128], mybir.dt.float32, space="PSUM")
                            nc.tensor.transpose(
                                out=tp[:], in_=stage[:, q * F + h * 128 : q * F + (h + 1) * 128],
                                identity=ident[:],
                            )
                            dst = xt_lo if h == 0 else xt_hi
                            nc.any.tensor_copy(out=dst[:, q * 128 : (q + 1) * 128], in_=tp[:])
                    xts.append((xt_lo, xt_hi))
                for b in range(B):
                    h, off = divmod(b, 2)
                    off *= 64
                    wt = wlo_t if off == 0 else whi_t
                    po = popool.tile([128, 512], mybir.dt.float32, space="PSUM")
                    for k in range(K):
                        xt = xts[k][h]
                        nc.tensor.matmul(
                            out=po[:, :nv], lhsT=wt[off : off + 64, k * COUT : (k + 1) * COUT],
                            rhs=xt[off : off + 64, :nv], start=(k == 0), stop=(k == K - 1),
                        )
                    ob = obpool.tile([128, 512], f32, tag="ob")
                    nc.any.tensor_scalar_add(ob[:, :nv], po[:, :nv], bias_t[:, 0:1])
                    nc.sync.dma_start(out_t[b, :, v0 : v0 + nv], ob[:, :nv])

    nc.compile()
    _CACHE[key] = nc
    return nc


# ---------------- entry point ----------------

def kernel(x, lap_rows, lap_cols, lap_vals, weight, bias):
    from concourse.bass_utils import run_bass_kernel_spmd

    x = np.asarray(x, np.float32)
    weight = np.asarray(weight, np.float32)
    bias = np.asarray(bias, np.float32)
    in_maps, NLO, NHI = host_inputs(x, lap_rows, lap_cols, lap_vals, weight, bias)
    nc = build_module(NLO, NHI)
    res = run_bass_kernel_spmd(nc, in_maps, core_ids=list(range(NC)))
    out = np.empty((B, COUT, V), np.float32)
    for c in range(NC):
        out[:, :, c * VC : (c + 1) * VC] = res.results[c]["out"][:, :, :VC]
    return out


# revision 12
# speedup vs baseline: 7.8265x; 7.8265x over previous
"""ChebConv (K=4) Trainium2 kernel: 8-core row-sharded SpMM + dense contraction.

Dataflow per core (rows [c*6250, (c+1)*6250) padded to 6272):
  x0 table (50176, 256) f32 in DRAM (host-built, padded global ids).
  3 Chebyshev SpMM steps: per 128-row tile, dma_gather the edge columns'
  z-rows from the table (int16 idx, lo/hi split at 32768), then reduce into
  the tile's 128 rows with per-128-slot indicator matmuls on the PE
  (lhsT M[slot, row] = val*(row_local[slot]==row), fp32r), accumulating in
  PSUM. Recurrence x_k = (2L)x_{k-1} - x_{k-2} via DVE subtract against the
  SBUF-resident ping/pong slice. Slices are AllGather'd into the next step's
  full table. Contraction phase: PE-transpose x_k tiles, then
  out[b].T (Cout, v) = sum_k W_k.T @ x_k[b].T with stationary fp32r weights.
"""

import sys

sys.path.insert(0, "/opt/trn_rl_repo")

import numpy as np

V = 50000
E = 800000
B, CIN, COUT, K = 4, 64, 128, 4
NC = 8
VC = V // NC              # 6250
VCP = 6272                # 49*128 padded rows per core
TILES = VCP // 128        # 49
VPT = NC * VCP            # 50176 table rows
F = B * CIN               # 256
HI_BASE = 32768           # int16 index split


# ---------------- host-side preprocessing ----------------

def _remap_col(g):
    return (g // VC) * VCP + (g % VC)


def preprocess(rows, cols, vals):
    """Split edges per core/tile into lo/hi index halves, pad to uniform
    counts, emit gather indices + per-chunk indicator metadata."""
    rows = np.asarray(rows)
    cols_r = _remap_col(np.asarray(cols).astype(np.int64))
    vals = np.asarray(vals, dtype=np.float32)

    per_core = []
    max_lo = max_hi = 0
    for c in range(NC):
        lo_r, hi_r = c * VC, (c + 1) * VC
        m = (rows >= lo_r) & (rows < hi_r)
        r = rows[m] - lo_r
        cg = cols_r[m]
        vv = vals[m]
        t_of = r // 128
        tiles = []
        for t in range(TILES):
            sel = t_of == t
            rt, ct, vt = r[sel], cg[sel], vv[sel]
            isl = ct < HI_BASE
            tl = (rt[isl] - t * 128, ct[isl], vt[isl])
            th = (rt[~isl] - t * 128, ct[~isl] - HI_BASE, vt[~isl])
            max_lo = max(max_lo, len(tl[0]))
            max_hi = max(max_hi, len(th[0]))
            tiles.append((tl, th))
        per_core.append(tiles)

    NLO = -(-max_lo // 128) * 128
    NHI = -(-max_hi // 128) * 128
    CPT = (NLO + NHI) // 128          # chunks per tile
    NCH = TILES * CPT
    SPT16 = (NLO + NHI) // 16         # idx columns per tile

    cores = []
    for c in range(NC):
        idx = np.zeros((128, TILES * SPT16), dtype=np.int16)
        rl = np.zeros((128, NCH), dtype=np.float32)
        v1 = np.zeros((128, NCH), dtype=np.float32)
        for t in range(TILES):
            (rlo, clo, vlo), (rhi, chi, vhi) = per_core[c][t]
            # slot order within tile: lo edges, lo pads, hi edges, hi pads
            rr = np.zeros(NLO + NHI, np.float32)
            cc = np.zeros(NLO + NHI, np.int32)
            vv = np.zeros(NLO + NHI, np.float32)
            n = len(rlo)
            rr[:n], cc[:n], vv[:n] = rlo, clo, vlo
            nh = len(rhi)
            rr[NLO : NLO + nh], cc[NLO : NLO + nh], vv[NLO : NLO + nh] = rhi, chi, vhi
            # gather idx, 16-wrapped, replicated across the 8 Q7 groups
            w = cc.reshape(SPT16, 16).T.astype(np.int16)       # (16, SPT16)
            idx[:, t * SPT16 : (t + 1) * SPT16] = np.tile(w, (8, 1))
            # chunk metadata: slot i -> chunk i//128, partition i%128
            ch0 = t * CPT
            rl[:, ch0 : ch0 + CPT] = rr.reshape(CPT, 128).T
            v1[:, ch0 : ch0 + CPT] = vv.reshape(CPT, 128).T
        cores.append(dict(idx=idx, rl=rl, v1=v1, v2=2.0 * v1))
    return cores, NLO, NHI


def host_inputs(x, lap_rows, lap_cols, lap_vals, weight, bias):
    x0 = np.ascontiguousarray(np.transpose(x, (2, 0, 1)).reshape(V, F)).astype(np.float32)
    table0 = np.zeros((VPT, F), dtype=np.float32)
    for c in range(NC):
        table0[c * VCP : c * VCP + VC] = x0[c * VC : (c + 1) * VC]
    cores, NLO, NHI = preprocess(lap_rows, lap_cols, lap_vals)
    iota = np.broadcast_to(np.arange(128, dtype=np.float32)[None, :], (128, 128)).copy()
    wlo = np.zeros((128, K * COUT), np.float32)
    whi = np.zeros((128, K * COUT), np.float32)
    for k in range(K):
        wlo[0:64, k * COUT : (k + 1) * COUT] = weight[k]
        whi[64:128, k * COUT : (k + 1) * COUT] = weight[k]
    bias_t = np.asarray(bias, np.float32).reshape(128, 1)
    in_maps = []
    for c in range(NC):
        in_maps.append(
            dict(
                x0slice=np.ascontiguousarray(table0[c * VCP : (c + 1) * VCP]),
                idx=cores[c]["idx"],
                rl=cores[c]["rl"],
                v1=cores[c]["v1"],
                v2=cores[c]["v2"],
                iota=iota,
                wlo=wlo,
                whi=whi,
                bias=bias_t,
            )
        )
    return in_maps, NLO, NHI


# ---------------- device module ----------------

_CACHE = {}


def build_module(NLO, NHI):
    key = (NLO, NHI)
    if key in _CACHE:
        return _CACHE[key]
    from concourse import bass, mybir, bacc
    import concourse.tile as tile
    from concourse.masks import make_identity

    CPT = (NLO + NHI) // 128
    NCH = TILES * CPT
    SPT16 = (NLO + NHI) // 16
    BLO = NLO // 128
    f32, f32r, i16 = mybir.dt.float32, mybir.dt.float32r, mybir.dt.int16

    nc = bacc.Bacc("TRN2", target_bir_lowering=False, debug=False, num_devices=NC)

    x0slice = nc.dram_tensor("x0slice", [VCP, F], f32, kind="ExternalInput")
    idx_in = nc.dram_tensor("idx", [128, TILES * SPT16], i16, kind="ExternalInput")
    rl_in = nc.dram_tensor("rl", [128, NCH], f32, kind="ExternalInput")
    v1_in = nc.dram_tensor("v1", [128, NCH], f32, kind="ExternalInput")
    v2_in = nc.dram_tensor("v2", [128, NCH], f32, kind="ExternalInput")
    iota_in = nc.dram_tensor("iota", [128, 128], f32, kind="ExternalInput")
    wlo_in = nc.dram_tensor("wlo", [128, K * COUT], f32, kind="ExternalInput")
    whi_in = nc.dram_tensor("whi", [128, K * COUT], f32, kind="ExternalInput")
    bias_in = nc.dram_tensor("bias", [128, 1], f32, kind="ExternalInput")
    out_t = nc.dram_tensor("out", [B, COUT, VCP], f32, kind="ExternalOutput")

    with tile.TileContext(nc) as tc:
        with (
            tc.tile_pool(name="pers", bufs=1) as pers,
            tc.tile_pool(name="gpool", bufs=2) as gpool,
            tc.tile_pool(name="mval", bufs=4) as mvpool,
            tc.tile_pool(name="spmm_ps", bufs=2, space="PSUM") as pspool,
            tc.tile_pool(name="tp_ps", bufs=2, space="PSUM") as tppool,
            tc.tile_pool(name="out_ps", bufs=2, space="PSUM") as popool,
            tc.tile_pool(name="stage", bufs=3) as spool,
            tc.tile_pool(name="xt", bufs=5) as xtpool,
            tc.tile_pool(name="obuf", bufs=3) as obpool,
            tc.tile_pool(name="dram", bufs=1, space="DRAM") as dram,
        ):
            # persistent loads
            idx_t = pers.tile([128, TILES * SPT16], i16)
            nc.sync.dma_start(idx_t[:], idx_in[:])
            rl_t = pers.tile([128, NCH], f32)
            nc.sync.dma_start(rl_t[:], rl_in[:])
            v1_t = pers.tile([128, NCH], f32)
            nc.sync.dma_start(v1_t[:], v1_in[:])
            v2_t = pers.tile([128, NCH], f32)
            nc.sync.dma_start(v2_t[:], v2_in[:])
            iota_t = pers.tile([128, 128], f32)
            nc.sync.dma_start(iota_t[:], iota_in[:])
            wlo_t = pers.tile([128, K * COUT], f32r)
            nc.gpsimd.dma_start(wlo_t[:], wlo_in[:])
            whi_t = pers.tile([128, K * COUT], f32r)
            nc.gpsimd.dma_start(whi_t[:], whi_in[:])
            bias_t = pers.tile([128, 1], f32)
            nc.sync.dma_start(bias_t[:], bias_in[:])
            ident = pers.tile([128, 128], f32)
            make_identity(nc, ident[:])

            bounce = [dram.tile([VCP, F], f32, name=f"bounce{i}", tag=f"bounce{i}") for i in range(3)]
            tables = [dram.tile([VPT, F], f32, name=f"table{i+1}", tag=f"table{i+1}") for i in range(2)]
            x0b = dram.tile([VCP, F], f32, name="x0b", tag="x0b")
            table0 = dram.tile([VPT, F], f32, name="table0i", tag="table0i")
            nc.sync.dma_start(x0b[:], x0slice[:])
            nc.gpsimd.collective_compute(
                "AllGather", mybir.AluOpType.bypass,
                replica_groups=[list(range(NC))],
                ins=[x0b.opt()], outs=[table0.opt()],
            )

            # ---------- SpMM steps ----------
            for k in (1, 2, 3):
                src = table0 if k == 1 else tables[k - 2]
                vmeta = v1_t if k == 1 else v2_t
                prev_src = None if k == 1 else (x0slice if k == 2 else bounce[0])
                for t in range(TILES):
                    gt = gpool.tile([128, CPT * F], f32r, tag="G")
                    c0 = t * SPT16
                    # Q7 scratch caps one gather at ~1024 idxs; split into <=896 segs
                    segs = []
                    for base, n, hi in ((0, NLO, False), (BLO * 128, NHI, True)):
                        done = 0
                        while done < n:
                            m = min(896, n - done)
                            segs.append((base + done, m, hi))
                            done += m
                    for off, n, hi in segs:
                        sap = src[HI_BASE:, :] if hi else src[:]
                        nc.gpsimd.dma_gather(
                            out_ap=gt[:, off * 2 : (off + n) * 2].rearrange(
                                "p (j f) -> p j f", f=F),
                            in_ap=sap.bitcast(f32r),
                            idxs_ap=idx_t[:, c0 + off // 16 : c0 + (off + n) // 16],
                            num_idxs=n, num_idxs_reg=n, elem_size=F,
                            single_packet=False,
                        )
                    ps = pspool.tile([128, F], mybir.dt.float32, space="PSUM")
                    for j in range(CPT):
                        ch = t * CPT + j
                        mv = mvpool.tile([128, 128], f32r)
                        nc.any.tensor_scalar(
                            out=mv[:], in0=iota_t[:],
                            scalar1=rl_t[:, ch : ch + 1], scalar2=vmeta[:, ch : ch + 1],
                            op0=mybir.AluOpType.is_equal, op1=mybir.AluOpType.mult,
                        )
                        nc.tensor.matmul(
                            out=ps[:], lhsT=mv[:], rhs=gt[:, j * F : (j + 1) * F],
                            start=(j == 0), stop=(j == CPT - 1),
                        )
                    xo = spool.tile([128, F], f32, tag="xout")
                    if k == 1:
                        nc.any.tensor_copy(out=xo[:], in_=ps[:])
                    else:
                        xp = spool.tile([128, F], f32, tag="xprev")
                        nc.sync.dma_start(xp[:], prev_src[t * 128 : (t + 1) * 128, :])
                        nc.vector.tensor_tensor(out=xo[:], in0=ps[:], in1=xp[:], op=mybir.AluOpType.subtract)
                    nc.sync.dma_start(bounce[k - 1][t * 128 : (t + 1) * 128, :], xo[:])
                if k < 3:
                    nc.gpsimd.collective_compute(
                        "AllGather", mybir.AluOpType.bypass,
                        replica_groups=[list(range(NC))],
                        ins=[bounce[k - 1].opt()], outs=[tables[k - 1].opt()],
                    )

            # ---------- contraction ----------
            srcs = [x0slice, bounce[0], bounce[1], bounce[2]]
            vblocks = [(i * 512, 512) for i in range(VCP // 512)]
            if VCP % 512:
                vblocks.append((VCP // 512 * 512, VCP % 512))
            for v0, nv in vblocks:
                nq = nv // 128
                xts = []
                for k in range(K):
                    stage = spool.tile([128, 4 * F], f32, tag="stage")
                    nc.sync.dma_start(
                        stage[:, : nq * F].rearrange("p (q f) -> p q f", f=F),
                        srcs[k][v0 : v0 + nv, :].rearrange("(q p) f -> p q f", p=128),
                    )
                    xt_lo = xtpool.tile([128, 512], f32r, tag="xtlo")
                    xt_hi = xtpool.tile([128, 512], f32r, tag="xthi")
                    for q in range(nq):
                        for h in range(2):
                            tp = tppool.tile([128, 128], mybir.dt.float32, space="PSUM")
                            nc.tensor.transpose(
                                out=tp[:], in_=stage[:, q * F + h * 128 : q * F + (h + 1) * 128],
                                identity=ident[:],
                            )
                            dst = xt_lo if h == 0 else xt_hi
                            nc.any.tensor_copy(out=dst[:, q * 128 : (q + 1) * 128], in_=tp[:])
                    xts.append((xt_lo, xt_hi))
                for b in range(B):
                    h, off = divmod(b, 2)
                    off *= 64
                    wt = wlo_t if off == 0 else whi_t
                    po = popool.tile([128, 512], mybir.dt.float32, space="PSUM")
                    for k in range(K):
                        xt = xts[k][h]
                        nc.tensor.matmul(
                            out=po[:, :nv], lhsT=wt[off : off + 64, k * COUT : (k + 1) * COUT],
                            rhs=xt[off : off + 64, :nv], start=(k == 0), stop=(k == K - 1),
                        )
                    ob = obpool.tile([128, 512], f32, tag="ob")
                    nc.any.tensor_scalar_add(ob[:, :nv], po[:, :nv], bias_t[:, 0:1])
                    nc.sync.dma_start(out_t[b, :, v0 : v0 + nv], ob[:, :nv])

    nc.compile()
    _CACHE[key] = nc
    return nc


# ---------------- entry point ----------------

def kernel(x, lap_rows, lap_cols, lap_vals, weight, bias):
    from concourse.bass_utils import run_bass_kernel_spmd

    x = np.asarray(x, np.float32)
    weight = np.asarray(weight, np.float32)
    bias = np.asarray(bias, np.float32)
    in_maps, NLO, NHI = host_inputs(x, lap_rows, lap_cols, lap_vals, weight, bias)
    nc = build_module(NLO, NHI)
    res = run_bass_kernel_spmd(nc, in_maps, core_ids=list(range(NC)))
    out = np.empty((B, COUT, V), np.float32)
    for c in range(NC):
        out[:, :, c * VC : (c + 1) * VC] = res.results[c]["out"][:, :, :VC]
    return out
